# revision 8
# baseline (speedup 1.0000x reference)
"""MoNet layer Trainium2 kernel (data-parallel over batch on 8 NeuronCores).

Math (per batch b, node i, neighbor j, gaussian k):
  edge      = ~isnan(coord[b,i,j,0])
  a_k       = mu_rho[k]  (upstream bug: theta also uses mu_rho)
  cr_k      = 0.5/(1e-14+sig_rho[k]^2),  ct_k = 0.5/(1e-14+sig_theta[k]^2)
  ang       = min(d, |2pi-d|), d = |theta - a_k|
  w[b,i,j,k]= edge * exp(-cr_k (rho-a_k)^2 - ct_k ang^2)
  agg[b,i,k,f] = sum_j w[b,i,j,k] x[b,j,f]
  out[b,i,:]   = (agg.reshape(K*F) @ fc_W.T + fc_b) * mask[b,i]

Device strategy:
  - Host pre-transposes all layouts (rho/theta -> [j,(jc,b,i)] bf16 with
    non-edge rho := 1e4 sentinel; x -> x^T; fc_W -> fcwt[f,(k,o)]*(pi/4)).
  - Gaussian product via TWO Derivative_Erf activations per k:
      wt = DErf(|sct*th + sct(pi-a)| - sct*pi) = (2/sqrt(pi)) exp(-ct*ang^2)
      wr = DErf(scr*rho - scr*a)               = (2/sqrt(pi)) exp(-cr(rho-a)^2)
    (the (4/pi) factor is folded into fc_W on host; rho sentinel 1e4 makes
    wr exactly 0 on HW, killing non-edges).
  - DVE does the theta affine+abs as bf16 tensor_scalar ops (4x mode);
    the wt*wr multiply alternates DVE/Pool to balance engines.
  - fc fused: out[i,o] = sum_{j,k} w[j,(i)k] z[j,(k,o)], z = x @ fcwt (bf16).
  - out accumulated in PSUM as out^T [o=64, i=256] per b, transposed back.
"""

import numpy as np
import ml_dtypes

import concourse.bass as bass
import concourse.mybir as mybir
import concourse.tile as tile
from concourse.bass_utils import run_bass_kernel_spmd

mdt = mybir.dt
F32 = mdt.float32
BF16 = mdt.bfloat16
I16 = mdt.int16
ALU = mybir.AluOpType
AF = mybir.ActivationFunctionType
NPBF16 = ml_dtypes.bfloat16

B, N, K, F_IN, F_OUT = 32, 256, 25, 64, 64
NCORES = 8
BL = B // NCORES            # batches per core
BI = BL * N                 # flattened (b, i) free dim = 1024
PI = np.pi
ZCH = 400                   # z-phase PSUM chunk (4 x 400 = 1600 cols)

# Route/balance tuning:
#   k in CX_KS      -> exp-route: two custom DVE ops + ACT Exp
#   other k         -> erf-route: DVE affine/abs + 2x ACT DErf + mult
#   erf-route mult runs on Pool for k % POOL_MOD == 0, else DVE
N_CX = 7
POOL_MOD = 2
Z_COPIES_PER_SLOT = 3


def _register_custom_dve_ops():
    """Register the two fused DVE ops with concourse.dve_ops (idempotent).

    WRAP_SQ_MONET: out = wrap(in0 + s0 into [-s1, s1] by one period imm2)^2
    FMA_SQ_MONET:  out = in1*imm2 + (in0*s0 + s1)^2
    """
    from concourse import dve_ops as dops
    from concourse.dve_spec import C0, C1, C2, Spec, Src0, Src1, lower, sq
    from concourse.dve_uop import DveOpSpec

    if "WRAP_SQ_MONET" in dops._SUB_OPCODE_FOR_NAME:
        by_name = {op.name: op for op in dops.OPS}
        return by_name["WRAP_SQ_MONET"], by_name["FMA_SQ_MONET"]

    _y = Src0 + C0
    wrap_spec = Spec(
        body=sq(_y + C2 * ((_y < -C1) - (_y > C1))),
        reference=lambda in0, in1, s0, s1, imm2: (
            lambda y: ((y + imm2 * ((y < -s1).astype(np.float32)
                                    - (y > s1).astype(np.float32))) ** 2
                       ).astype(np.float32))(in0.astype(np.float32) + s0),
    )
    fma_spec = Spec(
        body=Src1 * C2 + sq(Src0 * C0 + C1),
        reference=lambda in0, in1, s0, s1, imm2: (
            in1.astype(np.float32) * imm2
            + (in0.astype(np.float32) * s0 + s1) ** 2).astype(np.float32),
    )
    made = []
    for name, spec, rd1 in (("WRAP_SQ_MONET", wrap_spec, False),
                            ("FMA_SQ_MONET", fma_spec, True)):
        row = max(dops._SUB_OPCODE_FOR_NAME.values()) + 1
        dops._SUB_OPCODE_FOR_NAME[name] = row
        shas = {}
        for ver in ("v3", "v4"):
            uops = lower(spec, ver=ver)
            shas[ver] = DveOpSpec(name=name, opcode=row, uops=uops,
                                  rd1_en=rd1).sha(ver)
        op = dops.DveOp(name=name, spec=spec, subdim=False, uops_sha=shas)
        dops.OPS.append(op)
        dops.CUSTOM_DVE_SPECS[name] = spec
        made.append(op)
    return tuple(made)


def _split_excess_waits(nc, max_waits=1):
    """This walrus build rejects instructions carrying more than one sync
    wait. Hoist extra waits onto NoOp instructions inserted just before the
    over-subscribed instruction (same engine => program order preserves
    semantics)."""
    for f in nc.m.functions:
        for bb in f.blocks:
            changed = False
            new = []
            for inst in bb.instructions:
                si = inst.sync_info
                if si is not None and si.on_wait and len(si.on_wait) > max_waits:
                    waits = list(si.on_wait)
                    extra, keep = waits[:-max_waits], waits[-max_waits:]
                    for i in range(0, len(extra), max_waits):
                        nop = mybir.InstNoOp(name=nc.get_next_instruction_name())
                        nop.engine = inst.engine
                        nop.sync_info = mybir.SyncInfo(
                            on_wait=extra[i:i + max_waits], on_update=[])
                        nc.register_instruction(nop)
                        new.append(nop)
                    inst.sync_info = mybir.SyncInfo(
                        on_wait=keep, on_update=list(si.on_update))
                    changed = True
                new.append(inst)
            if changed:
                bb.instructions = new


def _f(v):
    return float(np.float32(v))


CX_KS = frozenset(range(K - N_CX, K))


def build_program(consts, iters=1):
    """Per-core Bass program. consts: dict of per-k host scalars."""
    sct, b1, bneg, scr, br = (
        consts["sct"], consts["b1"], consts["bneg"], consts["scr"], consts["br"]
    )
    neg_a, ct = consts["neg_a"], consts["ct"]
    if CX_KS:
        WRAP_SQ, FMA_SQ = _register_custom_dve_ops()
    nc = bass.Bass("TRN2", target_bir_lowering=False, debug=False)

    ktab_ap = nc.dram_tensor("ktab", [128, 2 * K], F32, kind="ExternalInput").ap()
    rt_ap = nc.dram_tensor("rhoT", [128, 2 * BI], BF16, kind="ExternalInput").ap()
    tt_ap = nc.dram_tensor("thetaT", [128, 2 * BI], BF16, kind="ExternalInput").ap()
    xt_ap = nc.dram_tensor("xt", [BL, F_IN, N], BF16, kind="ExternalInput").ap()
    fcwt_ap = nc.dram_tensor("fcwt", [F_IN, K * F_OUT], BF16, kind="ExternalInput").ap()
    maskr_ap = nc.dram_tensor("maskr", [BL, F_OUT, N], F32, kind="ExternalInput").ap()
    fcb_ap = nc.dram_tensor("fcb", [F_OUT, 1], F32, kind="ExternalInput").ap()
    ident_ap = nc.dram_tensor("ident", [128, 128], F32, kind="ExternalInput").ap()
    out_ap = nc.dram_tensor("out", [BL, N, F_OUT], F32, kind="ExternalOutput").ap()

    with tile.TileContext(nc) as tc:
        import contextlib

        with contextlib.ExitStack() as ctx:
            persist = ctx.enter_context(tc.tile_pool(name="persist", bufs=1))
            zps = ctx.enter_context(tc.tile_pool(name="zps", bufs=2, space="PSUM"))
            trps = ctx.enter_context(tc.tile_pool(name="trps", bufs=2, space="PSUM"))
            outps = ctx.enter_context(tc.tile_pool(name="outps", bufs=1, space="PSUM"))
            work = ctx.enter_context(tc.tile_pool(name="work", bufs=2))
            wpool = ctx.enter_context(tc.tile_pool(name="wpool", bufs=3))
            epi = ctx.enter_context(tc.tile_pool(name="epi", bufs=2))

            for it in range(iters):
                # ---- inputs in ----
                ident = persist.tile([128, 128], F32, tag="ident")
                nc.sync.dma_start(ident[:], ident_ap[:])
                ktab = persist.tile([128, 2 * K], F32, tag="ktab")
                nc.sync.dma_start(ktab[:], ktab_ap[:])
                fcb = persist.tile([F_OUT, 1], F32, tag="fcb")
                nc.sync.dma_start(fcb[:], fcb_ap[:])
                fcwt = persist.tile([F_IN, K * F_OUT], BF16, tag="fcwt")
                nc.sync.dma_start(fcwt[:], fcwt_ap[:])
                masks = persist.tile([F_OUT, BL * N], F32, tag="masks")
                for b in range(BL):
                    nc.sync.dma_start(masks[:, b * N:(b + 1) * N], maskr_ap[b])
                xts = []
                for b in range(BL):
                    xt = persist.tile([F_IN, N], BF16, tag=f"xt{b}")
                    nc.sync.dma_start(xt[:], xt_ap[b])
                    xts.append(xt)
                rt = persist.tile([128, 2 * BI], BF16, tag="rt")
                nc.sync.dma_start(rt[:], rt_ap[:])
                tt = persist.tile([128, 2 * BI], BF16, tag="tt")
                nc.sync.dma_start(tt[:], tt_ap[:])

                # ---- phase A: z[b,jc][j=128,(k,o)=1600] = xT chunk @ fcwt ----
                # PE matmuls emitted up front; the PSUM->SBUF copies (DVE)
                # are interleaved into the k-loop below so the ACT/DVE
                # gaussian pipeline starts immediately.
                KO = K * F_OUT
                NG = (KO + ZCH - 1) // ZCH
                zsb = []
                zq = []  # pending copy thunks, one per (b, jc, g)
                for b in range(BL):
                    zb = []
                    for jc in range(2):
                        z = persist.tile([128, KO], BF16, tag=f"z{b}{jc}")
                        for g in range(NG):
                            lo = g * ZCH
                            hi = min(KO, lo + ZCH)
                            zp = zps.tile([128, ZCH], F32, tag="zp")
                            nc.tensor.matmul(
                                zp[:, : hi - lo],
                                xts[b][:, jc * 128:(jc + 1) * 128],
                                fcwt[:, lo:hi],
                                start=True, stop=True)
                            zq.append((g, z, lo, hi, zp))
                        zb.append(z)
                    zsb.append(zb)
                zq.sort(key=lambda t: t[0])
                zq = [t[1:] for t in zq]
                z_done = 0

                def emit_z_copies(n):
                    nonlocal z_done
                    for z, lo, hi, zp in zq[z_done:z_done + n]:
                        nc.vector.tensor_copy(z[:, lo:hi], zp[:, : hi - lo])
                    z_done = min(len(zq), z_done + n)

                def need_chunks_done(k):
                    # all copies for chunks overlapping cols [k*64,(k+1)*64)
                    g_hi = min(NG - 1, ((k + 1) * F_OUT - 1) // ZCH)
                    return 8 * (g_hi + 1)

                # ---- out^T accumulators [o=64, i=256] per b ----
                outp = [outps.tile([F_OUT, N], F32, tag=f"op{b}", name=f"op{b}_{it}")
                        for b in range(BL)]

                # ---- phase B: gaussian weights + accumulation ----
                # erf-route k's first (DErf act table), then exp-route k's
                # (exp table) - one table switch total.
                erf_ks = [k for k in range(K) if k not in CX_KS]
                ks_order = erf_ks + sorted(CX_KS)
                first = ks_order[0]
                last = ks_order[-1]
                for k in ks_order:
                    w = wpool.tile([128, 2 * BI], BF16, tag="w")
                    if k in CX_KS:
                        t = work.tile([128, 2 * BI], BF16, tag="t")
                        nc.vector._custom_dve(
                            WRAP_SQ, out=t[:], in0=tt[:],
                            s0=neg_a[k], s1=_f(PI), imm2=_f(2 * PI))
                        s = work.tile([128, 2 * BI], BF16, tag="s")
                        nc.vector._custom_dve(
                            FMA_SQ, out=s[:], in0=rt[:], in1=t[:],
                            s0=scr[k], s1=br[k], imm2=ct[k])
                        nc.scalar.activation(w[:], s[:], AF.Exp,
                                             bias=0.0, scale=-1.0)
                    else:
                        y = work.tile([128, 2 * BI], BF16, tag="y")
                        nc.vector.tensor_scalar(
                            y[:], tt[:], sct[k], b1[k], ALU.mult, ALU.add)
                        u = work.tile([128, 2 * BI], BF16, tag="u")
                        nc.vector.tensor_scalar(
                            u[:].bitcast(I16), y[:].bitcast(I16),
                            0x7FFF, None, ALU.bitwise_and)
                        wt = work.tile([128, 2 * BI], BF16, tag="wt")
                        nc.scalar.activation(wt[:], u[:], AF.Derivative_Erf,
                                             bias=ktab[:, 2 * k:2 * k + 1],
                                             scale=1.0)
                        wr = work.tile([128, 2 * BI], BF16, tag="wr")
                        nc.scalar.activation(wr[:], rt[:], AF.Derivative_Erf,
                                             bias=ktab[:, 2 * k + 1:2 * k + 2],
                                             scale=scr[k])
                        if k % POOL_MOD == 0:
                            nc.gpsimd.tensor_tensor(w[:], wt[:], wr[:], ALU.mult)
                        else:
                            nc.vector.tensor_tensor(w[:], wt[:], wr[:], ALU.mult)
                    emit_z_copies(max(Z_COPIES_PER_SLOT,
                                      need_chunks_done(k) - z_done))
                    for b in range(BL):
                        for jc in range(2):
                            nc.tensor.matmul(
                                outp[b][:],
                                zsb[b][jc][:, k * F_OUT:(k + 1) * F_OUT],
                                w[:, jc * BI + b * N: jc * BI + (b + 1) * N],
                                start=(k == first and jc == 0),
                                stop=(k == last and jc == 1))

                # ---- epilogue: bias + mask, transpose back, store ----
                for b in range(BL):
                    ot = epi.tile([F_OUT, N], F32, tag="ot")
                    nc.vector.scalar_tensor_tensor(
                        ot[:], outp[b][:], fcb[:, 0:1],
                        masks[:, b * N:(b + 1) * N],
                        ALU.add, ALU.mult)
                    for ih in range(2):
                        tp = trps.tile([128, F_OUT], F32, tag="trp")
                        nc.tensor.transpose(
                            tp[:], ot[:, ih * 128:(ih + 1) * 128],
                            ident[:F_OUT, :F_OUT])
                        osb = epi.tile([128, F_OUT], F32, tag="osb")
                        nc.scalar.copy(osb[:], tp[:])
                        nc.sync.dma_start(out_ap[b, ih * 128:(ih + 1) * 128], osb[:])

    _split_excess_waits(nc)
    return nc


def _host_consts(coords_mu, sigma_rho, sigma_theta):
    a = np.asarray(coords_mu, np.float64)[0]            # [K] (bug: mu_rho everywhere)
    sr = np.asarray(sigma_rho, np.float64)
    st = np.asarray(sigma_theta, np.float64)
    cr = 0.5 / (1e-14 + sr * sr)
    ct = 0.5 / (1e-14 + st * st)
    scr = np.sqrt(cr)
    sct = np.sqrt(ct)
    consts = {
        "sct": [_f(v) for v in sct],                    # y = sct*th + b1
        "b1": [_f(v) for v in sct * (PI - a)],
        "bneg": [_f(v) for v in -sct * PI],             # wt = DErf(u + bneg)
        "scr": [_f(v) for v in scr],                    # wr = DErf(scr*rho + br)
        "br": [_f(v) for v in -scr * a],
        "neg_a": [_f(v) for v in -a],                   # exp-route wrap shift
        "ct": [_f(v) for v in ct],                      # exp-route theta coeff
    }
    ktab = np.zeros((128, 2 * K), np.float32)
    ktab[:, 0::2] = np.asarray(consts["bneg"], np.float32)
    ktab[:, 1::2] = np.asarray(consts["br"], np.float32)
    return consts, ktab


def _prep_inputs(x, coord, mask, fc_W, fc_b, ktab):
    """Host-side layout prep (transposes, sentinel fill, bf16 casts)."""
    edge = ~np.isnan(coord[..., 0])
    rho = np.where(edge, coord[..., 0], np.float32(1.0e4))
    theta = np.where(edge, coord[..., 1], np.float32(0.0))

    def to_jt(arr):  # [BLc,N,NJ] -> [j=128,(jc,b,i)=2*BLc*N] per core c
        blc = arr.shape[0]
        t = arr.transpose(2, 0, 1).reshape(2, 128, blc * N)
        return np.ascontiguousarray(
            t.transpose(1, 0, 2).reshape(128, 2 * blc * N).astype(NPBF16))

    fcwt = np.ascontiguousarray(
        (fc_W.reshape(F_OUT, K, F_IN).transpose(2, 1, 0)
         .reshape(F_IN, K * F_OUT) * np.float32(PI / 4.0)).astype(NPBF16))
    fcb = np.ascontiguousarray(fc_b.reshape(F_OUT, 1).astype(np.float32))
    ident = np.eye(128, dtype=np.float32)

    in_maps = []
    for c in range(NCORES):
        sl = slice(c * BL, (c + 1) * BL)
        maskr = np.ascontiguousarray(
            np.broadcast_to(mask[sl][:, None, :],
                            (BL, F_OUT, N)).astype(np.float32))
        in_maps.append({
            "rhoT": to_jt(rho[sl]),
            "thetaT": to_jt(theta[sl]),
            "xt": np.ascontiguousarray(x[sl].transpose(0, 2, 1).astype(NPBF16)),
            "fcwt": fcwt, "maskr": maskr, "fcb": fcb, "ident": ident,
            "ktab": ktab,
        })
    return in_maps


_CACHE = {}


def kernel(**inputs):
    x = np.asarray(inputs["x"], np.float32)
    coord = np.asarray(inputs["coord"], np.float32)
    mask = np.asarray(inputs["mask"], np.float32)
    coords_mu = np.asarray(inputs["coords_mu"], np.float32)
    sigma_rho = np.asarray(inputs["sigma_rho"], np.float32)
    sigma_theta = np.asarray(inputs["sigma_theta"], np.float32)
    fc_W = np.asarray(inputs["fc_W"], np.float32)
    fc_b = np.asarray(inputs["fc_b"], np.float32)

    consts, ktab = _host_consts(coords_mu, sigma_rho, sigma_theta)

    key = tuple(tuple(v) for v in consts.values())
    if key not in _CACHE:
        _CACHE.clear()
        _CACHE[key] = build_program(consts)
    nc = _CACHE[key]

    in_maps = _prep_inputs(x, coord, mask, fc_W, fc_b, ktab)
    res = run_bass_kernel_spmd(nc, in_maps, core_ids=list(range(NCORES)))
    out = np.concatenate([res.results[c]["out"] for c in range(NCORES)], axis=0)
    return out.astype(np.float32)
